# revision 4
# baseline (speedup 1.0000x reference)
"""Trainium2 Bass kernel for nn_DecoderLayer (PCGCv2-style sparse decoder).

Strategy (8 NeuronCores): data-parallel over batch (2) x 4 z-slabs of the
40^3 upsampled grid. Each core runs an identical SPMD program computing a
26-plane slab (10 output planes + halo-8 redundancy), entirely SBUF-resident:

  up-conv(2x2x2 stride2) -> 3^3 conv -> 3 InceptionResNet blocks -> 1x1 cls

Layout: feature maps as [channels(partitions), padded-space] where a 40x40
plane is stored 41x41 (shared zero pad row/col) so every 3^3 tap is a
constant AP offset (dz*1681 + dy*41 + dx). Convs are tap-wise fp32 matmul
accumulations in PSUM (27 taps + 1 bias tap against a ones-row). Occupancy
masking is fused into PSUM eviction (VectorE tensor_mul against a resident
replicated mask). Residual adds come straight from PSUM with zero-padded
M columns so all engine accesses are 32-aligned.

Arena rows: 0-63 h (phase A) / q01+o1b+dumps (blocks) / cls (final);
64-95 out (persistent); 96-127 replicated mask.

Host side: input slab/mask/weight packing, slab assembly, and the
adaptive top-k `keep` logic (argsort of 2x64000 scores - negligible).
"""
import os
if os.environ.get("JAX_PLATFORMS", "") == "cpu":
    # the device path runs the NEFF via PJRT on the axon platform; a
    # cpu-only pin (common for reference-side jax) would hide the cores
    os.environ["JAX_PLATFORMS"] = "axon,cpu"

import numpy as np
from contextlib import ExitStack

import concourse.bass as bass
import concourse.tile as tile
from concourse import bacc, mybir
from concourse import bass_utils

# ---- problem constants (hardcoded per spec) ----
B, CIN, HID, COUT, D, L = 2, 32, 64, 32, 20, 3
D2 = 2 * D
PLANE, ROWS, HEAD = 1681, 41, 42
NSLOT = 26
F = HEAD + NSLOT * PLANE + 42        # padded elems per channel row
NCORES = 8
RELU = mybir.ActivationFunctionType.Relu
F32 = mybir.dt.float32

TAPS = [((t // 9 - 1) * PLANE + ((t // 3) % 3 - 1) * ROWS + (t % 3 - 1))
        for t in range(27)]

# bias-table column offsets (btile [128, 328], content replicated per row)
B_CONV, B_A, B_B, B_C, B_CLS = 0, 32, 104, 224, 320   # A:24/blk, B:40/blk, C:32/blk
MA, MB, MC, MCLS = 24, 40, 32, 8                      # matmul M per stage


def _chunks(slot_lo, slot_hi):
    o0 = HEAD + slot_lo * PLANE
    o1 = HEAD + (slot_hi - 1) * PLANE + 40 * ROWS + 40 + 1
    res, o = [], o0
    while o < o1:
        n = min(512, o1 - o)
        res.append((o, n))
        o += n
    return res


def _build_program():
    nc = bacc.Bacc("TRN2", target_bir_lowering=False, debug=False,
                   enable_asserts=True, num_devices=NCORES)
    ap = {}
    ap["xs"] = nc.dram_tensor("xs", [33, 13 * 400], F32, kind="ExternalInput").ap()
    ap["mrep"] = nc.dram_tensor("mrep", [32, F], F32, kind="ExternalInput").ap()
    ap["wup"] = nc.dram_tensor("wup", [33, 8 * 64], F32, kind="ExternalInput").ap()
    ap["wconv"] = nc.dram_tensor("wconv", [64, 27 * 32], F32, kind="ExternalInput").ap()
    ap["wa"] = nc.dram_tensor("wa", [32, L * 27 * MA], F32, kind="ExternalInput").ap()
    ap["wb"] = nc.dram_tensor("wb", [16, L * 27 * MB], F32, kind="ExternalInput").ap()
    ap["wc"] = nc.dram_tensor("wc", [8, L * MC], F32, kind="ExternalInput").ap()
    ap["wcls"] = nc.dram_tensor("wcls", [32, 27 * MCLS], F32, kind="ExternalInput").ap()
    ap["biases"] = nc.dram_tensor("biases", [128, 328], F32, kind="ExternalInput").ap()
    out_ap = nc.dram_tensor("out_slab", [32, 10 * 1600], F32, kind="ExternalOutput").ap()
    cls_ap = nc.dram_tensor("cls_slab", [1, 10 * 1600], F32, kind="ExternalOutput").ap()

    with tile.TileContext(nc) as tc:
        with ExitStack() as ctx:
            pool = ctx.enter_context(tc.tile_pool(name="sbuf", bufs=1))
            psum = ctx.enter_context(tc.tile_pool(name="psum", bufs=1, space="PSUM"))
            arena = pool.tile([128, F], F32)
            xw = pool.tile([64, 2 * 400], F32)       # rows 0-32 data, 33-63 zero
            wup = pool.tile([64, 8 * 64], F32)       # rows 0-32 data, 33-63 zero
            wconv = pool.tile([64, 27 * 32], F32)
            wa = pool.tile([96, L * 27 * MA], F32)   # rows 64:96
            wb = pool.tile([16, L * 27 * MB], F32)
            wc = pool.tile([40, L * MC], F32)        # rows 32:40
            wcls = pool.tile([96, 27 * MCLS], F32)   # rows 64:96
            ones = pool.tile([128, 512], F32)
            btile = pool.tile([128, 328], F32)

            nc.gpsimd.memset(arena[0:64, :], 0.0)
            nc.gpsimd.memset(arena[64:96, :], 0.0)
            nc.gpsimd.memset(ones[:], 1.0)
            nc.gpsimd.memset(xw[:], 0.0)
            nc.gpsimd.memset(wup[:], 0.0)
            nc.sync.dma_start(arena[96:128, :], ap["mrep"][:])
            nc.sync.dma_start(wup[0:33, :], ap["wup"][:])
            nc.sync.dma_start(wconv[:], ap["wconv"][:])
            nc.sync.dma_start(wa[64:96, :], ap["wa"][:])
            nc.sync.dma_start(wb[:], ap["wb"][:])
            nc.sync.dma_start(wc[32:40, :], ap["wc"][:])
            nc.sync.dma_start(wcls[64:96, :], ap["wcls"][:])
            nc.sync.dma_start(btile[:], ap["biases"][:])

            psup_pool = dict(tag="psup", bufs=2)
            ps_pool = dict(tag="ps512", bufs=5)

            # ---- phase A: up-conv (K=64 zero-padded from 33) -> h rows 0:64
            for zs in range(NSLOT):
                j, p = zs // 2, zs % 2
                sl = (j % 2) * 400
                if p == 0:
                    nc.sync.dma_start(xw[0:33, sl:sl + 400],
                                      ap["xs"][:, j * 400:(j + 1) * 400])
                for q in range(2):
                    for r in range(2):
                        ps = psum.tile([64, 400], F32, space="PSUM", **psup_pool)
                        par = 4 * p + 2 * q + r
                        nc.tensor.matmul(ps[:], wup[:, par * 64:(par + 1) * 64],
                                         xw[:, sl:sl + 400], start=True, stop=True)
                        base = HEAD + zs * PLANE + q * ROWS + r
                        dest = arena[0:64, base:base + 1640].rearrange(
                            "p (y x) -> p y x", x=ROWS)[:, 0::2, 0:40:2]
                        nc.scalar.activation(
                            dest, ps[:].rearrange("p (a b) -> p a b", a=20), RELU)

            # ---- W_conv: h -> out (rows 64:96), slots [1,25) ----
            for (o, n) in _chunks(1, 25):
                ps = psum.tile([64, 512], F32, space="PSUM", **ps_pool)
                for t in range(27):
                    nc.tensor.matmul(ps[0:32, 0:n], wconv[:, t * 32:(t + 1) * 32],
                                     arena[0:64, o + TAPS[t]:o + TAPS[t] + n],
                                     start=(t == 0), stop=False)
                nc.tensor.matmul(ps[0:32, 0:n], btile[0:1, B_CONV:B_CONV + 32],
                                 ones[0:1, 0:n], start=False, stop=True)
                nc.scalar.activation(ps[0:32, 0:n], ps[0:32, 0:n], RELU)
                nc.vector.tensor_mul(arena[64:96, o:o + n], ps[0:32, 0:n],
                                     arena[96:128, o:o + n])

            # ---- blocks ----
            for l in range(L):
                # MM-A: out -> q01 (o0 rows 0-7, o1 rows 8-15), relu+mask
                for (o, n) in _chunks(2 + l, 24 - l):
                    ps = psum.tile([64, 512], F32, space="PSUM", **ps_pool)
                    for t in range(27):
                        w = wa[64:96, (l * 27 + t) * MA:(l * 27 + t + 1) * MA]
                        nc.tensor.matmul(ps[0:MA, 0:n], w,
                                         arena[64:96, o + TAPS[t]:o + TAPS[t] + n],
                                         start=(t == 0), stop=False)
                    nc.tensor.matmul(ps[0:MA, 0:n],
                                     btile[64:65, B_A + l * MA:B_A + (l + 1) * MA],
                                     ones[64:65, 0:n], start=False, stop=True)
                    nc.scalar.activation(ps[0:32, 0:n], ps[0:32, 0:n], RELU)
                    nc.vector.tensor_mul(arena[0:32, o:o + n], ps[0:32, 0:n],
                                         arena[96:128, o:o + n])
                # MM-B (K=16): q01 -> o0b (psum rows 0-15) + o1b (rows 32-39)
                for (o, n) in _chunks(3 + l, 23 - l):
                    ps = psum.tile([64, 512], F32, space="PSUM", **ps_pool)
                    for t in range(27):
                        w = wb[:, (l * 27 + t) * MB:(l * 27 + t + 1) * MB]
                        nc.tensor.matmul(ps[0:MB, 0:n], w,
                                         arena[0:16, o + TAPS[t]:o + TAPS[t] + n],
                                         start=(t == 0), stop=False)
                    nc.tensor.matmul(ps[0:MB, 0:n],
                                     btile[0:1, B_B + l * MB:B_B + (l + 1) * MB],
                                     ones[0:1, 0:n], start=False, stop=True)
                    # o1b: relu + mask -> arena rows 32:64 (32-39 real)
                    nc.scalar.activation(ps[32:64, 0:n], ps[32:64, 0:n], RELU)
                    nc.vector.tensor_mul(arena[32:64, o:o + n], ps[32:64, 0:n],
                                         arena[96:128, o:o + n])
                    # o0b: mask in psum, then residual out += [o0b; zeros]
                    nc.vector.tensor_mul(ps[0:32, 0:n], ps[0:32, 0:n],
                                         arena[96:128, o:o + n])
                    nc.vector.tensor_add(arena[64:96, o:o + n], ps[0:32, 0:n],
                                         arena[64:96, o:o + n])
                # MM-C (1x1, K=8): o1b -> o1c (psum rows 16-31), mask, residual
                for (o, n) in _chunks(3 + l, 23 - l):
                    ps = psum.tile([64, 512], F32, space="PSUM", **ps_pool)
                    nc.tensor.matmul(ps[0:MC, 0:n], wc[32:40, l * MC:(l + 1) * MC],
                                     arena[32:40, o:o + n], start=True, stop=False)
                    nc.tensor.matmul(ps[0:MC, 0:n],
                                     btile[32:33, B_C + l * MC:B_C + (l + 1) * MC],
                                     ones[32:33, 0:n], start=False, stop=True)
                    nc.vector.tensor_mul(ps[0:32, 0:n], ps[0:32, 0:n],
                                         arena[96:128, o:o + n])
                    nc.vector.tensor_add(arena[64:96, o:o + n], ps[0:32, 0:n],
                                         arena[64:96, o:o + n])

            # ---- cls (M=8, col 0 real): out -> arena rows 0:32 (row 0) ----
            for (o, n) in _chunks(8, 18):
                ps = psum.tile([64, 512], F32, space="PSUM", **ps_pool)
                for t in range(27):
                    nc.tensor.matmul(ps[0:MCLS, 0:n],
                                     wcls[64:96, t * MCLS:(t + 1) * MCLS],
                                     arena[64:96, o + TAPS[t]:o + TAPS[t] + n],
                                     start=(t == 0), stop=False)
                nc.tensor.matmul(ps[0:MCLS, 0:n], btile[64:65, B_CLS:B_CLS + MCLS],
                                 ones[64:65, 0:n], start=False, stop=True)
                nc.vector.tensor_mul(arena[0:32, o:o + n], ps[0:32, 0:n],
                                     arena[96:128, o:o + n])

            # ---- output DMAs: slots [8,18) ----
            for i in range(10):
                base = HEAD + (8 + i) * PLANE
                v = arena[64:96, base:base + 1640].rearrange(
                    "p (y x) -> p y x", x=ROWS)[:, :, 0:40]
                nc.sync.dma_start(
                    out_ap.rearrange("p (z yx) -> p z yx", z=10)[:, i], v)
                v2 = arena[0:1, base:base + 1640].rearrange(
                    "p (y x) -> p y x", x=ROWS)[:, :, 0:40]
                nc.sync.dma_start(
                    cls_ap.rearrange("p (z yx) -> p z yx", z=10)[:, i], v2)

    nc.compile()
    return nc


_NC = None


def _get_program():
    global _NC
    if _NC is None:
        _NC = _build_program()
    return _NC


def _pack_weights(W_up, b_up, W_conv, b_conv, blk_W00, blk_b00, blk_W01, blk_b01,
                  blk_W10, blk_b10, blk_W11, blk_b11, blk_W12, blk_b12,
                  W_cls, b_cls):
    wup = np.zeros((33, 8 * 64), np.float32)
    for p in range(2):
        for q in range(2):
            for r in range(2):
                par = 4 * p + 2 * q + r
                wup[0:32, par * 64:(par + 1) * 64] = W_up[:, :, p, q, r]
                wup[32, par * 64:(par + 1) * 64] = b_up
    wconv = np.zeros((64, 27 * 32), np.float32)
    for t in range(27):
        a, b_, c = t // 9, (t // 3) % 3, t % 3
        wconv[:, t * 32:(t + 1) * 32] = W_conv[:, :, a, b_, c].T
    wa = np.zeros((32, L * 27 * MA), np.float32)
    wb = np.zeros((16, L * 27 * MB), np.float32)
    wc = np.zeros((8, L * MC), np.float32)
    for l in range(L):
        for t in range(27):
            a, b_, c = t // 9, (t // 3) % 3, t % 3
            col = (l * 27 + t) * MA
            wa[:, col:col + 8] = blk_W00[l][:, :, a, b_, c].T
            if (a, b_, c) == (1, 1, 1):
                wa[:, col + 8:col + 16] = blk_W10[l][:, :, 0, 0, 0].T
            col = (l * 27 + t) * MB
            wb[0:8, col:col + 16] = blk_W01[l][:, :, a, b_, c].T
            wb[8:16, col + 32:col + 40] = blk_W11[l][:, :, a, b_, c].T
        wc[:, l * MC + 16:l * MC + 32] = blk_W12[l][:, :, 0, 0, 0].T
    wcls = np.zeros((32, 27 * MCLS), np.float32)
    for t in range(27):
        a, b_, c = t // 9, (t // 3) % 3, t % 3
        wcls[:, t * MCLS:t * MCLS + 1] = W_cls[:, :, a, b_, c].T
    biases = np.zeros((1, 328), np.float32)
    biases[0, B_CONV:B_CONV + 32] = b_conv
    for l in range(L):
        biases[0, B_A + l * MA:B_A + l * MA + 8] = blk_b00[l]
        biases[0, B_A + l * MA + 8:B_A + l * MA + 16] = blk_b10[l]
        biases[0, B_B + l * MB:B_B + l * MB + 16] = blk_b01[l]
        biases[0, B_B + l * MB + 32:B_B + l * MB + 40] = blk_b11[l]
        biases[0, B_C + l * MC + 16:B_C + l * MC + 32] = blk_b12[l]
    biases[0, B_CLS:B_CLS + 1] = b_cls
    biases = np.ascontiguousarray(np.broadcast_to(biases, (128, 328)))
    return dict(wup=wup, wconv=wconv, wa=wa, wb=wb, wc=wc, wcls=wcls,
                biases=biases)


def _core_inputs(x_feat, x_occ, occ_up, b, s, shared):
    zorig = 10 * s - 8
    occf = x_occ[b].astype(np.float32)
    xs = np.zeros((33, 13 * 400), np.float32)
    izlo = zorig // 2
    for j in range(13):
        iz = izlo + j
        if 0 <= iz < D:
            xs[0:32, j * 400:(j + 1) * 400] = \
                (x_feat[b, :, iz] * occf[iz]).reshape(32, 400)
            xs[32, j * 400:(j + 1) * 400] = occf[iz].ravel()
    mvec = np.zeros(F, np.float32)
    for slot in range(NSLOT):
        z = zorig + slot
        if 0 <= z < D2:
            mvec[HEAD + slot * PLANE:HEAD + slot * PLANE + 1640].reshape(
                40, ROWS)[:, 0:40] = occ_up[b, z]
    mrep = np.ascontiguousarray(np.broadcast_to(mvec[None, :], (32, F)))
    return dict(xs=xs, mrep=mrep, **shared)


def run_device(inputs, trace=False):
    """Run the 8-core SPMD program; returns (out_full, cls_full, results)."""
    x_feat = np.asarray(inputs["x_feat"], np.float32)
    x_occ = np.asarray(inputs["x_occ"])
    occ_up = np.repeat(np.repeat(np.repeat(x_occ, 2, 1), 2, 2), 2, 3) \
        .astype(np.float32)
    shared = _pack_weights(
        np.asarray(inputs["W_up"], np.float32), np.asarray(inputs["b_up"], np.float32),
        np.asarray(inputs["W_conv"], np.float32), np.asarray(inputs["b_conv"], np.float32),
        np.asarray(inputs["blk_W00"], np.float32), np.asarray(inputs["blk_b00"], np.float32),
        np.asarray(inputs["blk_W01"], np.float32), np.asarray(inputs["blk_b01"], np.float32),
        np.asarray(inputs["blk_W10"], np.float32), np.asarray(inputs["blk_b10"], np.float32),
        np.asarray(inputs["blk_W11"], np.float32), np.asarray(inputs["blk_b11"], np.float32),
        np.asarray(inputs["blk_W12"], np.float32), np.asarray(inputs["blk_b12"], np.float32),
        np.asarray(inputs["W_cls"], np.float32), np.asarray(inputs["b_cls"], np.float32))
    in_maps = []
    for c in range(NCORES):
        b, s = c // 4, c % 4
        in_maps.append(_core_inputs(x_feat, x_occ, occ_up, b, s, shared))
    nc = _get_program()
    import time as _time
    _t0 = _time.time()
    res = bass_utils.run_bass_kernel_spmd(nc, in_maps, core_ids=list(range(NCORES)),
                                          trace=trace)
    global LAST_SPMD_SECONDS
    LAST_SPMD_SECONDS = _time.time() - _t0
    out_full = np.zeros((B, 32, D2, D2, D2), np.float32)
    cls_full = np.zeros((B, 1, D2, D2, D2), np.float32)
    for c in range(NCORES):
        b, s = c // 4, c % 4
        out_full[b, :, 10 * s:10 * s + 10] = \
            res.results[c]["out_slab"].reshape(32, 10, 40, 40)
        cls_full[b, 0, 10 * s:10 * s + 10] = \
            res.results[c]["cls_slab"].reshape(10, 40, 40)
    return out_full, cls_full, res


def _host_finish(inputs, out_full, cls_full):
    x_occ = np.asarray(inputs["x_occ"])
    occ_up = np.repeat(np.repeat(np.repeat(x_occ, 2, 1), 2, 2), 2, 3)
    target_occ = np.asarray(inputs["target_occ"])
    occ_b = occ_up.astype(bool)
    target = occ_b & target_occ.astype(bool)
    cls = cls_full[:, 0].reshape(B, -1)
    occ_flat = occ_b.reshape(B, -1)
    adaptive = int(np.asarray(inputs["adaptive"]))
    if adaptive:
        vals = np.where(occ_flat, cls, -np.inf).astype(np.float32)
        t_cnt = target_occ.astype(bool).reshape(B, -1).sum(-1)
        k = np.minimum(occ_flat.sum(-1),
                       (t_cnt.astype(np.float32) * 1.0).astype(np.int32))
        order = np.argsort(-vals, axis=-1, kind="stable")
        ranks = np.argsort(order, axis=-1, kind="stable")
        keep = (ranks < k[:, None]) & occ_flat
    else:
        keep = (cls > 0) & occ_flat
    keep = keep.reshape(occ_b.shape)
    keep = keep | target
    out_pruned = out_full * keep[:, None].astype(out_full.dtype)
    return out_pruned, cls_full, target, keep


def kernel(**inputs):
    out_full, cls_full, _ = run_device(inputs, trace=False)
    return _host_finish(inputs, out_full, cls_full)
